# revision 39
# baseline (speedup 1.0000x reference)
"""Multi-head causal self-attention on 8 Trainium2 NeuronCores.

Problem: B=2, S=2048, E=1024, H=16 heads (D=64), causal mask, f32 I/O.

Sharding: (batch x head-group) -> 8 cores. Core c handles batch b=c//4 and
4 heads h0=4*(c%4).. (column-parallel Q/K/V projections, local attention,
row-parallel partial output projection). The 4 partial outputs per batch are
summed on the host (the "all-reduce" of row-parallel TP), where the output
bias bo and the folded V-bias term (bv @ Wo.T, exact because softmax rows
sum to 1) are also added.

Device kernel layout choices (all matmuls bf16 with f32 PSUM accumulate):
  - Host pre-transposes activations/weights so the kernel never transposes:
      qhT/khT = Wq_h @ q[b].T  (projection emits [d, s] directly)
      scores^T [k, q] = khT.T-contract-qhT (contract over d=64)
      exp on ScalarE, attn^T [k, q] feeds AV as the moving operand:
      ctx^T [d, q] = matmul(lhsT=V_aug [k, 128], rhs=attn^T)
    where V_aug cols 64:128 are ones, so rows 64:127 of the AV psum are the
    softmax row-sums. Normalization is a DVE reciprocal (read straight from
    the psum sum rows) + mul.
  - log2(e)/sqrt(D) is folded into Wq/bq; ScalarE computes exp via
    ACTIVATE(Exp, scale=ln2) so scores live in the log2 domain (ready for a
    DVE exp2 offload).
  - Causal structure: only lower-triangular k-blocks are computed; the
    128-wide diagonal band is masked by a multiplicative [128,2,128] triu
    tile after exp (exact: exp(s)*0 == 0).
  - Head-pair batching: scores/attn tiles are [128, 2, 512] (two heads, two
    PSUM banks); one ACTIVATE per k-block covers both heads (amortizes the
    ~352-cycle ACT fixed overhead).
  - Software pipelining: QK of block i+1 is emitted between exp(i) and
    AV(i), so the PE streams QK while ScalarE computes exp and never stalls
    on the exp->AV dependency. q-superblocks run in descending order so the
    final output projection overlaps earlier attention.
"""

import os
import sys

for _p in ("/opt/trn_rl_repo",):
    if _p not in sys.path and os.path.isdir(_p):
        sys.path.insert(0, _p)

import numpy as np
import ml_dtypes

import concourse.bacc as bacc
from concourse import mybir
from concourse.tile import TileContext
from concourse.bass_utils import run_bass_kernel_spmd

BF16 = ml_dtypes.bfloat16
P = 128
B, S, E, H, D = 2, 2048, 1024, 16, 64
HPC = 4            # heads per core
DC = HPC * D       # 256 output dims per core per projection
NCORES = 8
QSUP = 512         # q-superblock (matmul free dim)
NSUP = S // QSUP   # 4
NKB = S // P       # 16 k-blocks
SCALE = float(np.sqrt(D))
LOG2E = float(np.log2(np.e))
LN2 = float(np.log(2.0))

AF = mybir.ActivationFunctionType
f32 = mybir.dt.float32
bf16 = mybir.dt.bfloat16

_CACHE = {}
LAST = {}


def _install_axon_profile_shim():
    """Provide antenv.axon_hooks (absent in this image) so
    run_bass_kernel_spmd(trace=True) can NTFF-profile via libaxon_pjrt.so."""
    try:
        import antenv.axon_hooks  # noqa: F401
        return
    except ImportError:
        pass
    import contextlib
    import ctypes
    import types

    import antenv

    state = {"hook": None, "tried": False}

    def _build_hook():
        so_path = "/opt/axon/libaxon_pjrt.so"
        if not os.path.exists(so_path):
            return None
        lib = ctypes.CDLL(so_path)
        if not hasattr(lib, "axon_start_nrt_profile"):
            return None
        lib.axon_start_nrt_profile.argtypes = [
            ctypes.POINTER(ctypes.c_int64),
            ctypes.c_size_t,
        ]
        lib.axon_start_nrt_profile.restype = ctypes.c_int64
        lib.axon_stop_nrt_profile.argtypes = [ctypes.c_char_p]
        lib.axon_stop_nrt_profile.restype = ctypes.c_int64

        @contextlib.contextmanager
        def _hook(output_dir, device_ids):
            import jax

            jax.devices()
            if device_ids:
                ids = (ctypes.c_int64 * len(device_ids))(*device_ids)
                rc = lib.axon_start_nrt_profile(ids, len(device_ids))
            else:
                rc = lib.axon_start_nrt_profile(None, 0)
            if rc != 0:
                raise RuntimeError(f"axon_start_nrt_profile rc={rc}")
            try:
                yield
            finally:
                n = lib.axon_stop_nrt_profile(str(output_dir).encode())
                if n < 0:
                    raise RuntimeError(f"axon_stop_nrt_profile rc={n}")
                print(f"profile: {n} file(s) written to {output_dir}")

        return _hook

    mod = types.ModuleType("antenv.axon_hooks")

    def set_axon_ntff_profile_hook(h):
        state["hook"] = h
        state["tried"] = True

    def get_axon_ntff_profile_hook():
        if not state["tried"]:
            state["hook"] = _build_hook()
            state["tried"] = True
        return state["hook"]

    mod.set_axon_ntff_profile_hook = set_axon_ntff_profile_hook
    mod.get_axon_ntff_profile_hook = get_axon_ntff_profile_hook
    sys.modules["antenv.axon_hooks"] = mod
    antenv.axon_hooks = mod


_install_axon_profile_shim()


def _build_nc(causal: bool):
    nc = bacc.Bacc(None, target_bir_lowering=False)

    xqT = nc.dram_tensor("xqT", [E, S], bf16, kind="ExternalInput")
    xkT = nc.dram_tensor("xkT", [E, S], bf16, kind="ExternalInput")
    xvT = nc.dram_tensor("xvT", [E, S], bf16, kind="ExternalInput")
    wqT = nc.dram_tensor("wqT", [P, 8, DC], bf16, kind="ExternalInput")
    wkT = nc.dram_tensor("wkT", [P, 8, DC], bf16, kind="ExternalInput")
    wvT = nc.dram_tensor("wvT", [P, 8, DC], bf16, kind="ExternalInput")
    woT = nc.dram_tensor("woT", [P, 2, E], bf16, kind="ExternalInput")
    bqk = nc.dram_tensor("bqk", [P, 4], f32, kind="ExternalInput")
    cmask = nc.dram_tensor("cmask", [P, 2, P], bf16, kind="ExternalInput")
    # bf16 partials: host upcasts and sums the 4 row-parallel partials
    out = nc.dram_tensor("out", [S, E], bf16, kind="ExternalOutput")

    with TileContext(nc) as tc:
        with (
            tc.tile_pool(name="consts", bufs=1) as consts,
            tc.tile_pool(name="xin", bufs=24) as xin,
            tc.tile_pool(name="acts", bufs=1) as acts,
            tc.tile_pool(name="attn", bufs=4) as attn,
            tc.tile_pool(name="norm", bufs=4) as norm,
            tc.tile_pool(name="osb", bufs=3) as osb,
            tc.tile_pool(name="stp", bufs=2, space="PSUM") as stp,
            tc.tile_pool(name="cpool", bufs=2, space="PSUM") as cpool,
        ):
            # ---- input DMAs, all issued up front ---------------------------
            wq_sb = consts.tile([P, 8, DC], bf16)
            wk_sb = consts.tile([P, 8, DC], bf16)
            wv_sb = consts.tile([P, 8, DC], bf16)
            wo_sb = consts.tile([P, 2, E], bf16)
            bqk_sb = consts.tile([P, 4], f32)
            nc.sync.dma_start(wq_sb, wqT[:])
            nc.sync.dma_start(bqk_sb[:], bqk[:])

            def load_x(xT):
                xr = xT.rearrange("(ko p) s -> ko p s", p=P)
                tiles = []
                for ko in range(8):
                    t = xin.tile([P, S], bf16, tag="xin", name=f"x_{xT.name}_{ko}")
                    nc.sync.dma_start(t, xr[ko])
                    tiles.append(t)
                return tiles

            # DMA order matches consumption order: Q-proj, V-proj, K-proj.
            # K-proj last means the xk DMA tail hides under V-proj and the
            # PE stream runs straight from projections into attention.
            xq_t = load_x(xqT)
            nc.sync.dma_start(wv_sb, wvT[:])
            xv_t = load_x(xvT)
            if causal:
                cm_sb = consts.tile([P, 2, P], bf16)
                nc.sync.dma_start(cm_sb[:], cmask[:])
            nc.sync.dma_start(wk_sb, wkT[:])
            nc.sync.dma_start(wo_sb, woT[:])
            xk_t = load_x(xkT)

            # ---- memsets + HAM warm-up + ACT table preload -----------------
            warm = consts.tile([P, QSUP], bf16)
            nc.vector.memset(warm[:], 0.0)
            dummy = consts.tile([P, 1], f32)
            nc.scalar.activation(dummy[:], warm[:, 0:1], AF.Exp, scale=LN2)
            # HAM warm-up: one chained accumulation group (all-zero result)
            # with a live consumer below, so walrus DCE cannot drop it.
            wp = stp.tile([P, 2, QSUP], f32, tag="ps2", name="warm_ps")
            for wi in range(10):
                nc.tensor.matmul(wp[:, 0, :], warm[:, 0:P], warm[:],
                                 start=(wi == 0), stop=(wi == 9))

            # qhT/khT: per head-PAIR tiles [128, 2, S], zero-padded so every
            # QK matmul contracts over a full K=128 (HAM counts full-array
            # activity). Even heads carry data in partitions 0:64, odd heads
            # in 64:128; the complementary half stays zero. Separate tiles
            # per pair give tile-granular deps: attention on pair 0 can
            # start while pair 1's projection bias-adds still run.
            qh_p = []
            kh_p = []
            for pm in range(2):
                qt = acts.tile([P, 2, S], bf16, name=f"qh_p{pm}")
                kt = acts.tile([P, 2, S], bf16, name=f"kh_p{pm}")
                nc.gpsimd.memset(qt[:], 0.0)
                nc.gpsimd.memset(kt[:], 0.0)
                qh_p.append(qt)
                kh_p.append(kt)
            # V natural layout + ones block: [:, sb, h, 0:64] = vh, 64:128 ones
            vha = acts.tile([P, NKB, HPC, 2 * D], bf16)
            ctxT = acts.tile([P, 2, S], bf16)
            nc.vector.memset(vha[:, :, :, D:], 1.0)
            # warm-up liveness sink: writes exactly 1.0 (Copy(wp*0 + 1))
            # into one ones-column of vha, keeping the warm matmul chain
            # alive through DCE with zero numerical effect.
            nc.scalar.activation(vha[:, 0, 0, D:D + 1], wp[:, 0, 0:1],
                                 AF.Copy, bias=1.0, scale=0.0)

            # ---- projections (order: Q, V, K) -----------------------------
            # ko outer / chain inner: stationary w tile reused by 4
            # consecutive matmuls. The 4 s-superblock accumulation chains
            # live as two [128, 2, 512] pair tiles (one per psum pool).
            def qk_proj(pj, xt, w_sb, bcol, dst_p):
                for m in range(2):
                    cA = cpool.tile([P, 2, QSUP], f32, tag="cp2",
                                    name=f"pjA_{pj}_{m}")
                    cB = stp.tile([P, 2, QSUP], f32, tag="ps2",
                                  name=f"pjB_{pj}_{m}")
                    chains = [(cA, 0), (cA, 1), (cB, 0), (cB, 1)]
                    for ko in range(8):
                        for ns in range(NSUP):
                            ct, half = chains[ns]
                            nc.tensor.matmul(
                                ct[:, half, :],
                                w_sb[:, ko, m * P:(m + 1) * P],
                                xt[ko][:, ns * QSUP:(ns + 1) * QSUP],
                                start=(ko == 0),
                                stop=(ko == 7),
                            )
                    for ct, nsb in ((cA, 0), (cB, 2)):
                        nsl = slice(nsb * QSUP, (nsb + 2) * QSUP)
                        src = ct.rearrange("p a b -> p (a b)")
                        nc.vector.tensor_scalar_add(
                            dst_p[m][0:D, 0, nsl], src[0:D],
                            bqk_sb[0:D, bcol + m:bcol + m + 1],
                        )
                        nc.vector.tensor_scalar_add(
                            dst_p[m][D:, 1, nsl], src[D:],
                            bqk_sb[D:, bcol + m:bcol + m + 1],
                        )

            qk_proj(0, xq_t, wq_sb, 0, qh_p)

            for sb in range(NKB):
                pool = cpool if (sb & 1) else stp
                tag = "cp2" if (sb & 1) else "ps2"
                ps = pool.tile([P, 2, QSUP], f32, tag=tag, name=f"vp_{sb}")
                for ko in range(8):
                    nc.tensor.matmul(
                        ps[:, 0, 0:DC],
                        xv_t[ko][:, sb * P:(sb + 1) * P],
                        wv_sb[:, ko, :],
                        start=(ko == 0),
                        stop=(ko == 7),
                    )
                nc.vector.tensor_copy(
                    vha[:, sb, :, 0:D],
                    ps[:, 0, 0:DC].rearrange("p (h d) -> p h d", h=HPC),
                )

            qk_proj(1, xk_t, wk_sb, 2, kh_p)

            # ---- attention (software-pipelined) ---------------------------
            # Units = (qs, m) head-pair segments, qs descending. Iteration
            # (unit, kb): QK pair -> exp pair -> [diag mask] -> AV pair.
            # QK runs one iteration ahead of AV so the PE streams QK(i+1)
            # while ScalarE computes exp(i).
            # All m=0 units first: one long pure QK/exp/AV stream with no
            # norm-at-m1/outproj interleave (keeps the PE stream dense and
            # the HAM clock up); the m=1 half carries the outproj spreading.
            # m=0 half descending (big units while the clock ramps); m=1
            # half ASCENDING so the attention ends on the longest densest
            # unit — the HAM clock stays up into the tail where the final
            # outproj otherwise runs at half clock.
            units = [(qs, 0) for qs in (3, 2, 1, 0)] + \
                    [(qs, 1) for qs in (0, 1, 2, 3)]
            iters = []
            for u, (qs, m) in enumerate(units):
                nkb = 4 * qs + 4 if causal else NKB
                for kb in range(nkb):
                    iters.append((u, qs, m, kb, nkb))

            st_of = {}
            at_of = {}
            cps_of = {}

            def emit_qk(i):
                u, qs, m, kb, nkb = iters[i]
                r = kb - 4 * qs
                qlo = r * P if (causal and r >= 0) else 0
                st = stp.tile([P, 2, QSUP], f32, tag="ps2",
                              name=f"st_{u}_{kb}")
                for h2 in range(2):
                    h = 2 * m + h2
                    nc.tensor.matmul(
                        st[:, h2, qlo:],
                        kh_p[m][:, h2, kb * P:(kb + 1) * P],
                        qh_p[m][:, h2, qs * QSUP + qlo:(qs + 1) * QSUP],
                        start=True, stop=True,
                    )
                st_of[i] = (st, qlo)

            last_at = {}

            def emit_exp_mask(i):
                u, qs, m, kb, nkb = iters[i]
                st, qlo = st_of[i]
                at = attn.tile([P, 2, QSUP], bf16, tag="at",
                               name=f"at_{u}_{kb}")
                last_at["t"] = at
                nc.scalar.activation(at[:, :, qlo:], st[:, :, qlo:],
                                     AF.Exp, scale=LN2)
                if causal and kb - 4 * qs >= 0:
                    nc.vector.tensor_mul(
                        at[:, :, qlo:qlo + P], at[:, :, qlo:qlo + P], cm_sb,
                    )
                at_of[i] = (at, qlo)

            def emit_av(i):
                u, qs, m, kb, nkb = iters[i]
                at, qlo = at_of.pop(i)
                st_of.pop(i, None)
                if kb == 0:
                    cps_of[u] = cpool.tile([P, 2, QSUP], f32, tag="cp2",
                                           name=f"cps_{u}")
                cps = cps_of[u]
                for h2 in range(2):
                    h = 2 * m + h2
                    nc.tensor.matmul(
                        cps[:, h2, qlo:],
                        vha[:, kb, h, :],
                        at[:, h2, qlo:],
                        start=(kb == 0), stop=(kb == nkb - 1),
                    )

            # Deferred work queue: one item per attention iteration. Keeps
            # unit boundaries down to 4 partition-aligned DVE copies so the
            # QK/exp/AV stream never stalls (stalls flip the PE into
            # exposed-LDWEIGHTS mode, +~100ns on every matmul).
            deferred = []

            def emit_norm_deferred(u):
                # The r4-proven norm ops (copy-down sums, aligned recip,
                # cross-up mul), time-shifted into the next unit so the
                # boundary itself emits nothing on the DVE. The cps psum
                # pair stays live one extra unit; its pool slot is not
                # reallocated until one unit after that, so no hazard.
                qs, m = units[u]
                cps = cps_of.pop(u)

                def fin():
                    for h2 in range(2):
                        sums = norm.tile([D, QSUP], f32, tag="sums",
                                         name=f"sums_{u}_{h2}")
                        nc.vector.tensor_copy(out=sums, in_=cps[D:, h2, :])
                        rec = norm.tile([D, QSUP], f32, tag="rec",
                                        name=f"rec_{u}_{h2}")
                        nc.vector.reciprocal_approx_fast(out=rec, in_=sums)
                        nc.vector.tensor_mul(
                            ctxT[h2 * D:(h2 + 1) * D, m,
                                 qs * QSUP:(qs + 1) * QSUP],
                            cps[0:D, h2, :], rec)

                deferred.append(fin)

            # Output projection: ONE reused cpool tile per qs; its 4 per-sb
            # chains (matmuls -> copy -> DMA) run as deferred items in the
            # following unit, so no extra cpool allocations sit between
            # consecutive AV-chain (cps) allocations and the PE never waits
            # on a pso-copy drain.
            tail_state = {}

            def emit_chunk_body(pso, sb, warm_cont=False, scalar_copy=False):
                for km in range(2):
                    for n2 in range(2):
                        # warm_cont: bank n2=0 continues the tail warm-up
                        # accumulation group (which added exactly 0.0)
                        st_flag = (km == 0) and not (warm_cont and n2 == 0)
                        nc.tensor.matmul(
                            pso[:, n2, :],
                            ctxT[:, km, sb * P:(sb + 1) * P],
                            wo_sb[:, km, n2 * QSUP:(n2 + 1) * QSUP],
                            start=st_flag, stop=(km == 1),
                        )
                ot = osb.tile([P, 2, QSUP], bf16, tag="ot", name=f"ot_{sb}")
                if scalar_copy:
                    # tail: ScalarE is idle after the last exp; ACT Copy
                    # drains psum in parallel with the DVE so the pso
                    # ping-pong never stalls on a copy
                    nc.scalar.activation(
                        ot.rearrange("p a b -> p (a b)"),
                        pso.rearrange("p a b -> p (a b)"),
                        AF.Copy,
                    )
                else:
                    nc.vector.tensor_copy(
                        out=ot.rearrange("p a b -> p (a b)"),
                        in_=pso.rearrange("p a b -> p (a b)"),
                    )
                nc.sync.dma_start(
                    out[sb * P:(sb + 1) * P, :],
                    ot.rearrange("p a b -> p (a b)"),
                )

            def start_outproj(qs):
                if qs == 3:
                    # tail outproj: two psos ping-pong (no copy-wait gaps);
                    # pso "b" allocated lazily inside its first chunk, after
                    # the final norm has been emitted (pool-slot safety)
                    def tchunk(sb, j):
                        def emit():
                            if (j & 1) and "b" not in tail_state:
                                tail_state["b"] = cpool.tile(
                                    [P, 2, QSUP], f32, tag="cp2",
                                    name="pso3b")
                            pso = (tail_state["a"] if (j & 1) == 0
                                   else tail_state["b"])
                            emit_chunk_body(pso, sb, warm_cont=(j == 0),
                                            scalar_copy=True)
                        return emit

                    for j, sb in enumerate(range(12, 16)):
                        deferred.append(tchunk(sb, j))
                    return

                pso = cpool.tile([P, 2, QSUP], f32, tag="cp2",
                                 name=f"pso_{qs}")

                def chunk(sb):
                    def emit():
                        emit_chunk_body(pso, sb)
                    return emit

                for sb in range(4 * qs, 4 * qs + 4):
                    deferred.append(chunk(sb))

            n_it = len(iters)
            emit_qk(0)
            for i in range(n_it):
                u, qs, m, kb, nkb = iters[i]
                emit_exp_mask(i)
                # pop BEFORE emit_av: the deferred fin(u-1) must be emitted
                # before emit_av's cps allocation takes a rotated pool slot
                if deferred:
                    deferred.pop(0)()
                if i + 1 < n_it:
                    emit_qk(i + 1)
                emit_av(i)
                if kb == nkb - 1:
                    if m == 1:
                        # flush stale chunks before the new pso allocation
                        # takes the previous pso's cpool slot
                        while deferred:
                            deferred.pop(0)()
                    emit_norm_deferred(u)
                    if m == 1:
                        start_outproj(qs)
            # Tail clock keep-alive: 6 zero-result matmuls (zero STATIONARY
            # x last attention tile as moving operand -- the data dependency
            # on the final exp pins them to the tail against scheduler
            # reordering) into the tail pso's n2=0 bank with stop=False; the
            # first tail chunk's km chain continues the group with
            # start=False, accumulating exactly 0.0. Keeps the PE busy
            # through the final norm's DVE latency so the HAM clock stays up
            # for the last outproj, and is DCE-proof via real dataflow.
            tail_state["a"] = cpool.tile([P, 2, QSUP], f32, tag="cp2",
                                         name="pso3a")
            for wi in range(12):
                nc.tensor.matmul(tail_state["a"][:, 0, :], warm[:, 0:P],
                                 last_at["t"][:, 0, :],
                                 start=(wi == 0), stop=False)
            while deferred:
                deferred.pop(0)()

    nc.finalize()
    return nc


def _get_nc(causal: bool):
    key = ("nc", causal)
    if key not in _CACHE:
        _CACHE[key] = _build_nc(causal)
    return _CACHE[key]


def _bf(a):
    return np.ascontiguousarray(a, dtype=np.float32).astype(BF16)


def _wperm(wT, nko):
    """[nko*128, M] -> [128, nko, M] so each SBUF partition's data is one
    contiguous run in DRAM (single DMA descriptor per partition)."""
    wT = np.asarray(wT, np.float32)
    m = wT.shape[1]
    return np.ascontiguousarray(
        wT.reshape(nko, P, m).transpose(1, 0, 2)).astype(BF16)


def kernel(q, k, v, mask, Wq, bq, Wk, bk, Wv, bv, Wo, bo):
    q = np.asarray(q, np.float32)
    k = np.asarray(k, np.float32)
    v = np.asarray(v, np.float32)
    mask = np.asarray(mask)
    Wq, bq = np.asarray(Wq, np.float32), np.asarray(bq, np.float32)
    Wk, bk = np.asarray(Wk, np.float32), np.asarray(bk, np.float32)
    Wv, bv = np.asarray(Wv, np.float32), np.asarray(bv, np.float32)
    Wo, bo = np.asarray(Wo, np.float32), np.asarray(bo, np.float32)

    m2 = mask.reshape(S, S) != 0
    if m2.all():
        causal = False
    else:
        tri = np.tril(np.ones((S, S), bool))
        assert (m2 == tri).all(), "only causal or all-ones masks supported"
        causal = True

    nc = _get_nc(causal)

    cm1 = np.asarray(
        np.arange(P)[:, None] <= np.arange(P)[None, :], np.float32
    ).astype(BF16)  # [k, q] keep-region of the diagonal 128-band
    cm = np.ascontiguousarray(
        np.broadcast_to(cm1[:, None, :], (P, 2, P))).astype(BF16)

    xT = {}
    for b in range(B):
        xT[("q", b)] = _bf(q[b].T)
        xT[("k", b)] = _bf(k[b].T)
        xT[("v", b)] = _bf(v[b].T)

    # log2(e)/sqrt(D) folded into Wq/bq: scores come out in the log2 domain
    qscale = LOG2E / SCALE
    in_maps = []
    for c in range(NCORES):
        b = c // 4
        rows = slice((c % 4) * DC, (c % 4) * DC + DC)
        bq_s = (bq[rows] * qscale).reshape(2, P).T
        bk_s = bk[rows].reshape(2, P).T
        in_maps.append({
            "xqT": xT[("q", b)],
            "xkT": xT[("k", b)],
            "xvT": xT[("v", b)],
            "wqT": _wperm(Wq[rows].T * qscale, 8),
            "wkT": _wperm(Wk[rows].T, 8),
            "wvT": _wperm(Wv[rows].T, 8),
            "woT": _wperm(Wo[:, rows].T, 2),
            "bqk": np.ascontiguousarray(
                np.concatenate([bq_s, bk_s], axis=1), np.float32),
            "cmask": cm,
        })

    res = run_bass_kernel_spmd(nc, in_maps, core_ids=list(range(NCORES)))
    LAST["exec_time_ns"] = res.exec_time_ns
    LAST["results"] = res

    host_bias = (bo + bv @ Wo.T).astype(np.float32)
    out = np.zeros((B, S, E), np.float32)
    for c in range(NCORES):
        out[c // 4] += res.results[c]["out"].astype(np.float32)
    out += host_bias
    return out
